# revision 40
# baseline (speedup 1.0000x reference)
"""Trainium2 Bass kernel for DGM_c batch problem.

Math (per the reference):
  xe = x @ W                                   [N, 32]
  centroid/scale from graph 0's embedding (detached)
  xg = (xe - centroid) * scale                 per graph [n, 32]
  D  = pairwise squared euclidean dists        [G, n, n]
  A  = sigmoid(temperature * (|threshold| - D))
  outputs: xe, dense edge_index (deterministic pattern), edge_weight = A.ravel()

Sharding: graph-axis data parallel over 8 cores, 4 graphs each. Graph-0 rows
(x0) are replicated so every core computes centroid/scale locally (no
collectives). Key kernel trick: D is produced by ONE fp32 matmul per
[128, 512] output block using augmented operands with K=33:
  L = [z (32 rows); -0.5],  R = [z (32 rows); sq_n]
  (L.T @ R)[m, n] = z_m . z_n - 0.5 sq_n
followed by a single ScalarE sigmoid with runtime per-partition scale
(2*temperature) and per-m-block bias (temperature*|threshold| -
temperature*sq_m), reading PSUM directly.

Edge indices are generated on GPSIMD (otherwise idle) from iota patterns plus
a runtime per-core row-base offset, and DMA'd out as large contiguous stores.
Emission order keeps the compute-critical path (transposes -> xeT -> L/R ->
Gram -> sigmoid) early in each engine's in-order stream; index generation and
the small xe output ride in the gaps.
"""

import numpy as np
from contextlib import ExitStack

import concourse.bacc as bacc
import concourse.bass as bass
import concourse.mybir as mybir
import concourse.tile as tile
from concourse.bass_utils import run_bass_kernel_spmd
from concourse.masks import make_identity

G = 32          # graphs
NPG = 1024      # nodes per graph
F = 128         # input features
EMB = 32        # embedding dim
NCORES = 8
GPC = G // NCORES           # graphs per core = 4
NS = GPC * NPG              # shard rows = 4096
dt = mybir.dt
ts = bass.ts

_CACHE = {}


def _emit(ctx, tc, io):
    nc = tc.nc
    AF = mybir.ActivationFunctionType
    ALU = mybir.AluOpType
    AX = mybir.AxisListType
    f32 = dt.float32
    i32 = dt.int32

    singles = ctx.enter_context(tc.tile_pool(name="singles", bufs=1))
    lrp = ctx.enter_context(tc.tile_pool(name="lrp", bufs=4))
    ewp = ctx.enter_context(tc.tile_pool(name="ewp", bufs=4))
    sip = ctx.enter_context(tc.tile_pool(name="sip", bufs=3))
    dip = ctx.enter_context(tc.tile_pool(name="dip", bufs=1))
    xep = ctx.enter_context(tc.tile_pool(name="xep", bufs=2))
    wkp = ctx.enter_context(tc.tile_pool(name="wkp", bufs=2))
    ps2 = ctx.enter_context(tc.tile_pool(name="ps2", bufs=2, space="PSUM"))
    psd = ctx.enter_context(tc.tile_pool(name="psd", bufs=3, space="PSUM"))
    pst = ctx.enter_context(tc.tile_pool(name="pst", bufs=1, space="PSUM"))

    # ---- one packed leading input: x0 (partition-major) | W | params ----
    # x0wp[p, :1024] = x0 rows (t*128+p), x0wp[p, 1024:1056] = W[p],
    # x0wp[0, 1056:1059] = (temp, thr, rowbase). One DMA, no setup gaps.
    NX0 = 8 * F
    x0wt = singles.tile([F, NX0 + EMB + 3], f32, name="x0wt")
    nc.sync.dma_start(out=x0wt, in_=io["x0wp"])
    w_sb = x0wt[:, NX0:NX0 + EMB]
    temp_sb = x0wt[0:1, NX0 + EMB:NX0 + EMB + 1]
    thr_sb = x0wt[0:1, NX0 + EMB + 1:NX0 + EMB + 2]
    rowb_sb = x0wt[0:1, NX0 + EMB + 2:NX0 + EMB + 3]
    # rbase broadcast to 128 partitions via a K=1 ones-matmul on PE (idle at
    # kernel start; avoids both POOL's in-order stream and an extra tiny DMA)
    ones1x128 = singles.tile([1, 128], f32, name="ones1x128")
    nc.gpsimd.memset(ones1x128, 1.0)
    rb_ps = pst.tile([128, 1], f32, tag="tiny", name="rb_ps")
    nc.tensor.matmul(rb_ps, lhsT=ones1x128, rhs=rowb_sb)
    rbase128 = singles.tile([128, 1], f32, name="rbase128")
    nc.vector.tensor_copy(rbase128, rb_ps)
    # di values for all 4 graphs in one [128, 4*1024] tile (the master iota is
    # POOL's first op); each graph stored by one 4MB DMA whose read AP repeats
    # the row block 8x (step-0 middle dim). iota_gj borrows an si ring slot
    # (same footprint, dead before si generation peaks).
    iota_gj = sip.tile([128, 4, 1024], f32, tag="si", name="iota_gj")
    nc.gpsimd.iota(iota_gj, pattern=[[1024, 4], [1, 1024]], base=0,
                   channel_multiplier=0, allow_small_or_imprecise_dtypes=True)
    di_all = dip.tile([128, 4, 1024], i32, tag="di", name="di_all")
    nc.vector.tensor_scalar(di_all[:, 0:2, :], iota_gj[:, 0:2, :], rbase128,
                            None, op0=ALU.add)
    nc.vector.tensor_scalar(di_all[:, 2:4, :], iota_gj[:, 2:4, :], rbase128,
                            None, op0=ALU.add)
    di_view = io["di"].rearrange("(g t p) j -> g p t j", t=8, p=128)
    for g in range(GPC):
        dg = di_all[:, g, :]
        di_rep = bass.AP(tensor=dg.tensor, offset=dg.offset,
                         ap=[dg.ap[0], [0, 8], dg.ap[-1]])
        nc.sync.dma_start(out=di_view[g], in_=di_rep)

    # ---- constants + ALL input DMAs (inputs first in every DMA queue) ----
    identity = singles.tile([128, 128], f32, name="identity")
    make_identity(nc, identity)
    ones128 = singles.tile([128, 1], f32, name="ones128")
    nc.gpsimd.memset(ones128, 1.0)
    ones32 = singles.tile([32, 1], f32, name="ones32")
    nc.gpsimd.memset(ones32, 1.0)
    onesr = singles.tile([1, 32], f32, name="onesr")
    nc.gpsimd.memset(onesr, 1.0)
    # xn shares the ew staging ring (same 16KB/partition footprint, disjoint
    # lifetime: xn dies after the transposes, before ew staging peaks)
    xn = ewp.tile([128, 32, F], f32, tag="ew", name="xn")
    xs_r = io["xs"].rearrange("(t p) f -> p t f", p=128)
    for q in range(2):
        nc.sync.dma_start(out=xn[:, ts(q, 16), :], in_=xs_r[:, ts(q, 16), :])

    # hoist the sigmoid ACT table load to kernel start (dummy op)
    dummy_sg = singles.tile([1, 1], f32, name="dummy_sg")
    nc.scalar.activation(dummy_sg, temp_sb, AF.Sigmoid)

    # augmented operand tiles, allocated up front (all 4 graphs resident so
    # no slot-reuse waits serialize the fill pipeline). Plain fp32: fp32r
    # would make the Gram matmuls 4x faster on PE but costs ~40x accuracy
    # (2.5e-4 vs 6e-6 measured on HW) for only ~3% end-to-end, since the
    # kernel is DMA-bound.
    Ls, Rs = [], []
    for g in range(GPC):
        Lg = lrp.tile([33, NPG], f32, tag="L", name=f"L_{g}")
        Rg = lrp.tile([33, NPG], f32, tag="R", name=f"R_{g}")
        nc.vector.memset(Lg[32:33, :], -0.5)
        Ls.append(Lg)
        Rs.append(Rg)

    # runtime sigmoid scale/bias scalars
    athr = singles.tile([1, 1], f32, name="athr")
    nc.scalar.activation(athr, thr_sb, AF.Abs)
    bias11 = singles.tile([1, 1], f32, name="bias11")
    nc.vector.tensor_mul(bias11, temp_sb, athr)
    bias128 = singles.tile([128, 1], f32, name="bias128")
    nc.gpsimd.partition_broadcast(bias128, bias11)
    t2_11 = singles.tile([1, 1], f32, name="t2_11")
    nc.vector.tensor_scalar_mul(t2_11, temp_sb, 2.0)
    s128 = singles.tile([128, 1], f32, name="s128")
    nc.gpsimd.partition_broadcast(s128, t2_11)
    nt_11 = singles.tile([1, 1], f32, name="nt_11")
    nc.vector.tensor_scalar_mul(nt_11, temp_sb, -1.0)
    negt128 = singles.tile([128, 1], f32, name="negt128")
    nc.gpsimd.partition_broadcast(negt128, nt_11)

    # ---- src edge indices on GPSIMD (the bulk filler work) ----
    # iota_tp borrows an ew-ring slot (same 16KB/partition footprint; dead
    # after the last si generation, before ew staging peaks)
    iota_tp = ewp.tile([128, 4, 1024], f32, tag="ew", name="iota_tp")
    nc.gpsimd.iota(iota_tp, pattern=[[128, 4], [0, 1024]], base=0,
                   channel_multiplier=1, allow_small_or_imprecise_dtypes=True)
    si_view = io["si"].rearrange("(b t p) j -> b p t j", t=4, p=128)
    for g in range(GPC):
        for h in range(2):
            cv = singles.tile([128, 1], f32, name=f"cv_si_{g}_{h}")
            nc.gpsimd.iota(cv, pattern=[[0, 1]], base=g * NPG + h * 512,
                           channel_multiplier=0,
                           allow_small_or_imprecise_dtypes=True)
            bg = singles.tile([128, 1], f32, name=f"bg_si_{g}_{h}")
            nc.gpsimd.tensor_add(bg, cv, rbase128)
            si_t = sip.tile([128, 4, 1024], i32, tag="si", name=f"si_{g}_{h}")
            # alternate engines: POOL and DVE generate src indices in parallel
            eng = nc.vector if (g * 2 + h) % 2 else nc.gpsimd
            eng.tensor_scalar(si_t, iota_tp, bg, None, op0=ALU.add)
            nc.sync.dma_start(out=si_view[g * 2 + h], in_=si_t)

    # ---- prelim: centroid & scale from graph 0 (replicated on all cores) ----
    # centroid = (sum_rows x0) @ W / 1024
    cf_ps = pst.tile([128, 1], f32, tag="tiny", name="cf_ps")
    for t in range(8):
        nc.tensor.matmul(cf_ps, lhsT=x0wt[:, ts(t, 128)], rhs=ones128,
                         start=(t == 0), stop=(t == 7))
    cf_sb = singles.tile([128, 1], f32, name="cf_sb")
    nc.scalar.mul(cf_sb, cf_ps, 1.0 / NPG)
    cent_ps = pst.tile([32, 1], f32, tag="tiny", name="cent_ps")
    nc.tensor.matmul(cent_ps, lhsT=w_sb, rhs=cf_sb)
    cent_sb = singles.tile([32, 1], f32, name="cent_sb")
    nc.scalar.copy(cent_sb, cent_ps)

    # x0 transposed -> [F, 1024]
    x0T = singles.tile([128, NPG], f32, name="x0T")
    for t in range(8):
        tp_ps = ps2.tile([128, 128], f32, tag="tp", name=f"tp0_{t}")
        nc.tensor.transpose(tp_ps, x0wt[:, ts(t, 128)], identity)
        nc.scalar.copy(x0T[:, ts(t, 128)], tp_ps)

    # max|xe0 - centroid| over all elements -> scale vec
    m0 = singles.tile([32, NPG], f32, name="m0")
    for h in range(2):
        e0_ps = ps2.tile([32, 512], f32, tag="b512", name=f"e0_{h}")
        nc.tensor.matmul(e0_ps, lhsT=w_sb, rhs=x0T[:, ts(h, 512)])
        nc.vector.tensor_scalar(m0[:, ts(h, 512)], e0_ps, cent_sb, None,
                                op0=ALU.subtract)
    maxv = singles.tile([32, 1], f32, name="maxv")
    nc.vector.reduce_max(maxv, m0, axis=AX.X, apply_absolute_value=True)
    # cross-partition max without touching POOL's in-order stream (which is
    # busy generating edge indices): 32x32 DVE stream-transpose, row-reduce,
    # then K=1 ones-matmul to broadcast the scalar back to 32 partitions.
    mx = singles.tile([32, 32], f32, name="mx")
    nc.vector.memset(mx, 0.0)
    nc.vector.tensor_copy(mx[:, 0:1], maxv)
    mxT = singles.tile([32, 32], f32, name="mxT")
    nc.vector.transpose(mxT, mx)
    gmax1 = singles.tile([1, 1], f32, name="gmax1")
    nc.vector.reduce_max(gmax1, mxT[0:1, :], axis=AX.X)
    bc_ps = pst.tile([32, 1], f32, tag="tiny", name="bc_ps")
    nc.tensor.matmul(bc_ps, lhsT=onesr, rhs=gmax1)
    r32 = singles.tile([32, 1], f32, name="r32")
    nc.vector.reciprocal(r32, bc_ps)
    scale32 = singles.tile([32, 1], f32, name="scale32")
    nc.vector.tensor_scalar_mul(scale32, r32, 0.9)
    negc32 = singles.tile([32, 1], f32, name="negc32")
    nc.vector.tensor_mul(negc32, cent_sb, scale32)
    nc.vector.tensor_scalar_mul(negc32, negc32, -1.0)

    # ---- main shard pipeline ----
    xT = singles.tile([128, NS], f32, name="xT")
    for t in range(32):
        tp_ps = ps2.tile([128, 128], f32, tag="tp", name=f"tp_{t}")
        nc.tensor.transpose(tp_ps, xn[:, t, :], identity)
        nc.vector.tensor_copy(xT[:, ts(t, 128)], tp_ps)

    # fill L = [z; -0.5], R = [z; sq]; per-m-block sigmoid bias vectors
    # (L.T @ R)[m, n] = z_m . z_n - 0.5 sq_n; the -0.5 sq_m term is folded
    # into the per-partition sigmoid bias (bias_blk = t*|thr| - t*sq_m).
    bias_blks = []
    for g in range(GPC):
        Lg, Rg = Ls[g], Rs[g]
        for h in range(2):
            c = g * 2 + h
            et_ps = ps2.tile([32, 512], f32, tag="b512", name=f"et_{c}")
            nc.tensor.matmul(et_ps, lhsT=w_sb, rhs=xT[:, ts(c, 512)])
            nc.vector.tensor_scalar(Lg[0:32, ts(h, 512)], et_ps, scale32,
                                    negc32, op0=ALU.mult, op1=ALU.add)
            nc.vector.tensor_scalar(Rg[0:32, ts(h, 512)], et_ps, scale32,
                                    negc32, op0=ALU.mult, op1=ALU.add)
            z2 = wkp.tile([32, 512], f32, tag="z2", name=f"z2_{c}")
            zin = Lg[0:32, ts(h, 512)]
            nc.vector.tensor_mul(z2, zin, zin)
            sq_ps = ps2.tile([1, 512], f32, tag="b512", name=f"sq_{c}")
            nc.tensor.matmul(sq_ps, lhsT=ones32, rhs=z2)
            nc.scalar.copy(Rg[32:33, ts(h, 512)], sq_ps)
            # node-major sq for the sigmoid bias of the 4 m-blocks in chunk h
            for mb in range(4):
                m = h * 4 + mb
                sqt_ps = ps2.tile([128, 1], f32, tag="tp", name=f"sqt_{c}_{mb}")
                nc.tensor.matmul(sqt_ps, lhsT=z2[:, ts(mb, 128)], rhs=ones32)
                bb = singles.tile([128, 1], f32, name=f"bb_{g}_{m}")
                nc.vector.tensor_scalar(bb, sqt_ps, negt128, bias128,
                                        op0=ALU.mult, op1=ALU.add)
                bias_blks.append(bb)

    # ---- distance blocks -> sigmoid -> edge weights ----
    ew_view = io["ew"].rearrange("(g v m p) j -> g v p m j", v=2, m=4, p=128)
    for g in range(GPC):
        for v in range(2):
            ew_t = ewp.tile([128, 4, 1024], f32, tag="ew", name=f"ew_{g}_{v}")
            for m2 in range(4):
                m = v * 4 + m2
                for h in range(2):
                    d_ps = psd.tile([128, 512], f32, tag="d",
                                    name=f"d_{g}_{m}_{h}")
                    nc.tensor.matmul(d_ps, lhsT=Ls[g][:, ts(m, 128)],
                                     rhs=Rs[g][:, ts(h, 512)])
                    nc.scalar.activation(ew_t[:, m2, ts(h, 512)], d_ps,
                                         AF.Sigmoid,
                                         bias=bias_blks[g * 8 + m], scale=s128)
            nc.sync.dma_start(out=ew_view[g, v], in_=ew_t)

    # ---- xe output: xe = x @ W, one staged tile, one DMA ----
    xe_view = io["xe"].rearrange("(t p) e -> p t e", p=128)
    xe_t = xep.tile([128, 32, EMB], f32, tag="xe", name="xe_t")
    for t in range(32):
        xp_ps = ps2.tile([128, EMB], f32, tag="tp", name=f"xp_{t}")
        nc.tensor.matmul(xp_ps, lhsT=xT[:, ts(t, 128)], rhs=w_sb)
        nc.scalar.copy(xe_t[:, t, :], xp_ps)
    nc.sync.dma_start(out=xe_view, in_=xe_t)


def _build_nc():
    if "nc" in _CACHE:
        return _CACHE["nc"]
    nc = bacc.Bacc("TRN2", target_bir_lowering=False, debug=False,
                   num_devices=NCORES)
    io = {
        "xs": nc.dram_tensor("xs", [NS, F], dt.float32, kind="ExternalInput").ap(),
        "x0wp": nc.dram_tensor("x0wp", [F, 8 * F + EMB + 3], dt.float32,
                               kind="ExternalInput").ap(),
        "xe": nc.dram_tensor("xe", [NS, EMB], dt.float32, kind="ExternalOutput").ap(),
        "ew": nc.dram_tensor("ew", [NS, NPG], dt.float32, kind="ExternalOutput").ap(),
        "si": nc.dram_tensor("si", [NS, NPG], dt.int32, kind="ExternalOutput").ap(),
        "di": nc.dram_tensor("di", [NS, NPG], dt.int32, kind="ExternalOutput").ap(),
    }
    with tile.TileContext(nc) as tc:
        with ExitStack() as ctx:
            _emit(ctx, tc, io)
    nc.compile()
    _CACHE["nc"] = nc
    return nc


def make_in_maps(x, W, temperature, threshold):
    x = np.ascontiguousarray(np.asarray(x, dtype=np.float32))
    W = np.ascontiguousarray(np.asarray(W, dtype=np.float32))
    t11 = np.asarray(temperature, dtype=np.float32).reshape(1, 1)
    h11 = np.asarray(threshold, dtype=np.float32).reshape(1, 1)
    # x0 in partition-major layout: x0p[p, t*128:...] = x[t*128+p, :]
    x0p = x[:NPG].reshape(8, 128, F).transpose(1, 0, 2).reshape(F, 8 * F)
    in_maps = []
    for c in range(NCORES):
        x0wp = np.zeros((F, 8 * F + EMB + 3), dtype=np.float32)
        x0wp[:, :8 * F] = x0p
        x0wp[:, 8 * F:8 * F + EMB] = W
        x0wp[0, 8 * F + EMB:] = (t11[0, 0], h11[0, 0], float(c * NS))
        in_maps.append({
            "xs": np.ascontiguousarray(x[c * NS:(c + 1) * NS]),
            "x0wp": x0wp,
        })
    return in_maps


def assemble(results):
    NE = G * NPG * NPG
    xe = np.empty((G * NPG, EMB), dtype=np.float32)
    ew = np.empty(NE, dtype=np.float32)
    ei = np.empty((2, NE), dtype=np.int32)
    per = NS * NPG
    for c, r in enumerate(results):
        xe[c * NS:(c + 1) * NS] = r["xe"]
        ew[c * per:(c + 1) * per] = r["ew"].ravel()
        ei[0, c * per:(c + 1) * per] = r["si"].ravel()
        ei[1, c * per:(c + 1) * per] = r["di"].ravel()
    return xe, ei, ew


def kernel(x, W, temperature, threshold, batch, edge_index, num_graphs,
           **run_kwargs):
    nc = _build_nc()
    in_maps = make_in_maps(x, W, temperature, threshold)
    res = run_bass_kernel_spmd(nc, in_maps, core_ids=list(range(NCORES)),
                               **run_kwargs)
    out = assemble(res.results)
    if run_kwargs:
        kernel.last_result = res
    return out
